# revision 8
# baseline (speedup 1.0000x reference)
import contextlib
import ctypes
import os
import sys
import types

import numpy as np

B, S, D = 2, 2048, 1024
H, HD = 16, 64
N_CORES = 8
TP = 4
HPC = H // TP
QCOLS = HPC * HD
ESL = D // TP
KC = D // 128
NKT = S // 128
NQT = S // 512
GROUPS = [[0, 1, 2, 3], [4, 5, 6, 7]]


def _install_ntff_shim():
    if "antenv.axon_hooks" in sys.modules:
        return
    try:
        lib = ctypes.CDLL("/opt/axon/libaxon_pjrt.so")
        lib.axon_start_nrt_profile.argtypes = [ctypes.POINTER(ctypes.c_int64), ctypes.c_size_t]
        lib.axon_start_nrt_profile.restype = ctypes.c_int64
        lib.axon_stop_nrt_profile.argtypes = [ctypes.c_char_p]
        lib.axon_stop_nrt_profile.restype = ctypes.c_int64
    except (OSError, AttributeError):
        lib = None

    @contextlib.contextmanager
    def _hook(output_dir, device_ids):
        import jax
        jax.devices()
        if device_ids:
            ids = (ctypes.c_int64 * len(device_ids))(*device_ids)
            rc = lib.axon_start_nrt_profile(ids, len(device_ids))
        else:
            rc = lib.axon_start_nrt_profile(None, 0)
        if rc != 0:
            raise RuntimeError(f"axon_start_nrt_profile rc={rc}")
        try:
            yield
        finally:
            n = lib.axon_stop_nrt_profile(str(output_dir).encode())
            print(f"profile: {n} file(s) written to {output_dir}", file=sys.stderr)

    mod = types.ModuleType("antenv.axon_hooks")
    mod.get_axon_ntff_profile_hook = lambda: (_hook if lib is not None else None)
    mod.set_axon_ntff_profile_hook = lambda h: None
    sys.modules["antenv.axon_hooks"] = mod


_install_ntff_shim()

import concourse.bacc as bacc
import concourse.mybir as mybir
import concourse.tile as tile
from concourse.bass_utils import run_bass_kernel_spmd

F32 = mybir.dt.float32
BF16 = mybir.dt.bfloat16
NPBF16 = np.dtype(mybir.dt.np(BF16))
EXP = mybir.ActivationFunctionType.Exp
MUL = mybir.AluOpType.mult
ADD = mybir.AluOpType.add


def build_graph():
    nc = bacc.Bacc("TRN2", target_bir_lowering=False, debug=False,
                   enable_asserts=True, num_devices=N_CORES)

    xt_d = [nc.dram_tensor(f"xt{nt}", [128, KC * 512], BF16, kind="ExternalInput")
            for nt in range(NQT)]
    wqk_d = nc.dram_tensor("wqk", [D, 2 * QCOLS], BF16, kind="ExternalInput")
    wv_d = nc.dram_tensor("wv", [D, QCOLS], BF16, kind="ExternalInput")
    wp_d = nc.dram_tensor("wp", [D, ESL], BF16, kind="ExternalInput")
    bqk_d = nc.dram_tensor("bqk", [128, 4], F32, kind="ExternalInput")
    bv_d = nc.dram_tensor("bv", [128, QCOLS], F32, kind="ExternalInput")
    bp_d = nc.dram_tensor("bp", [128, ESL], F32, kind="ExternalInput")
    tri_d = nc.dram_tensor("tri", [128, 128], BF16, kind="ExternalInput")
    out_d = nc.dram_tensor("out", [S, ESL], F32, kind="ExternalOutput")

    with tile.TileContext(nc) as tc:
        with (
            tc.tile_pool(name="sb", bufs=1) as sb,
            tc.tile_pool(name="pt", bufs=6) as ptp,
            tc.tile_pool(name="ob", bufs=2) as obp,
            tc.tile_pool(name="rb", bufs=4) as rbp,
            tc.tile_pool(name="psS", bufs=2, space="PSUM") as psS,
            tc.tile_pool(name="psZ", bufs=2, space="PSUM") as psZ,
            tc.tile_pool(name="psA", bufs=2, space="PSUM") as psA,
            tc.tile_pool(name="dram", bufs=6, space="DRAM") as dram,
        ):
            xT_sb = sb.tile([128, KC * S], BF16, tag="xT")
            wqk_sb = sb.tile([128, KC * 2 * QCOLS], BF16, tag="wqk")
            wv_sb = sb.tile([128, KC * QCOLS], BF16, tag="wv")
            wp_sb = sb.tile([128, KC * ESL], BF16, tag="wp")
            qT_sb = sb.tile([128, 2 * S], BF16, tag="qT")
            kT_sb = sb.tile([128, 2 * S], BF16, tag="kT")
            v_sb = sb.tile([128, NKT * HPC * (HD + 1)], BF16, tag="v")
            z_sb = sb.tile([128, 2 * S], BF16, tag="z")
            zg_sb = sb.tile([128, KC * S], BF16, tag="zg")
            bqk_sb = sb.tile([128, 4], F32, tag="bqk")
            bv_sb = sb.tile([128, QCOLS], F32, tag="bv")
            bp_sb = sb.tile([128, ESL], F32, tag="bp")
            tri_sb = sb.tile([128, 128], BF16, tag="tri")
            onesb = sb.tile([33, HD], BF16, tag="onesb")
            scr = sb.tile([128, 512], BF16, tag="scr")

            nc.vector.memset(scr[:], 0.125)
            nc.vector.memset(onesb[:], 1.0)
            nc.vector.memset(v_sb[:], 1.0)

            def loadk(dst_sb, src_d, fs, k0, k1):
                nc.sync.dma_start(
                    out=dst_sb[:].rearrange("p (k s) -> p k s", k=KC)[:, k0:k1, :],
                    in_=src_d[:, :].rearrange("(k p) s -> p k s", p=128)[:, k0:k1, :])

            def load_xt_nt(nt):
                nc.sync.dma_start(
                    out=xT_sb[:, nt * KC * 512:(nt + 1) * KC * 512],
                    in_=xt_d[nt][:, :])

            loadk(wqk_sb, wqk_d, 512, 0, KC)
            load_xt_nt(0)
            nc.sync.dma_start(out=tri_sb[:], in_=tri_d[:])
            nc.sync.dma_start(out=bqk_sb[:], in_=bqk_d[:])
            loadk(wv_sb, wv_d, QCOLS, 0, KC)
            nc.sync.dma_start(out=bv_sb[:], in_=bv_d[:])
            load_xt_nt(1)
            load_xt_nt(2)
            load_xt_nt(3)
            nc.sync.dma_start(out=bp_sb[:], in_=bp_d[:])
            loadk(wp_sb, wp_d, ESL, 0, KC)

            warm = psA.tile([128, 512], F32, tag="m", name="warm")
            for _ in range(12):
                nc.tensor.matmul(warm[:], lhsT=scr[:, 0:128], rhs=scr[:],
                                 start=True, stop=True, skip_group_check=True)

            def qk_proj_nt(mc, nt):
                ps = psA.tile([128, 512], F32, tag="m")
                for k in range(KC):
                    nc.tensor.matmul(
                        ps[:],
                        lhsT=wqk_sb[:, k * 512 + mc * 128: k * 512 + (mc + 1) * 128],
                        rhs=xT_sb[:, (nt * KC + k) * 512:(nt * KC + k + 1) * 512],
                        start=(k == 0), stop=(k == KC - 1))
                dst = qT_sb if mc < 2 else kT_sb
                c2 = mc % 2
                nc.vector.tensor_scalar_add(
                    dst[:, c2 * S + nt * 512: c2 * S + (nt + 1) * 512],
                    ps[:], bqk_sb[:, mc:mc + 1])

            def v_proj(t):
                psv = psA.tile([128, QCOLS], F32, tag="m")
                for k in range(KC):
                    c0 = ((t // 4) * KC + k) * 512 + (t % 4) * 128
                    nc.tensor.matmul(
                        psv[:],
                        lhsT=xT_sb[:, c0:c0 + 128],
                        rhs=wv_sb[:, k * QCOLS:(k + 1) * QCOLS],
                        start=(k == 0), stop=(k == KC - 1))
                vdst = v_sb[:].rearrange(
                    "p (t h e) -> p t h e", t=NKT, e=HD + 1)[:, t, :, 0:HD]
                nc.vector.tensor_tensor(
                    vdst,
                    psv[:].rearrange("p (h d) -> p h d", h=HPC),
                    bv_sb[:].rearrange("p (h d) -> p h d", h=HPC),
                    ADD)

            pend = [None]
            zaug_box = [None, None]

            def normalize_and_gather(hp, qt, zaug0, zaug1):
                r_pack = rbp.tile([33, 512], F32, tag="rp")
                nc.vector.tensor_copy(r_pack[0:1, :], zaug0[HD:HD + 1, :])
                nc.vector.tensor_copy(r_pack[32:33, :], zaug1[HD:HD + 1, :])
                rinv = rbp.tile([33, 512], F32, tag="ri")
                nc.vector.reciprocal_approx_fast(out=rinv[:], in_=r_pack[0:33, :])
                rinvb = rbp.tile([33, 512], BF16, tag="rib")
                nc.vector.tensor_copy(rinvb[:], rinv[:])
                rb_ps = psA.tile([128, 512], F32, tag="m", name=f"rb_{hp}_{qt}")
                nc.tensor.matmul(rb_ps[0:64, :], lhsT=onesb[0:1, :],
                                 rhs=rinvb[0:1, :], start=True, stop=True,
                                 tile_position=(0, 0), skip_group_check=True)
                nc.tensor.matmul(rb_ps[64:128, :], lhsT=onesb[32:33, :],
                                 rhs=rinvb[32:33, :], start=True, stop=True,
                                 tile_position=(32, 64), skip_group_check=True)
                rb_sb = rbp.tile([128, 512], BF16, tag="rb")
                nc.vector.tensor_copy(rb_sb[:], rb_ps[:])
                zc = hp * S + qt * 512
                nc.vector.tensor_tensor(z_sb[0:64, zc:zc + 512],
                                        zaug0[0:HD, :], rb_sb[0:64, :], MUL)
                nc.vector.tensor_tensor(z_sb[64:128, zc:zc + 512],
                                        zaug1[0:HD, :], rb_sb[64:128, :], MUL)
                zd = dram.tile([128, 512], BF16, tag="zd")
                zgd = dram.tile([TP * 128, 512], BF16, tag="zgd")
                nc.sync.dma_start(out=zd[:], in_=z_sb[:, zc:zc + 512])
                nc.gpsimd.collective_compute(
                    "AllGather", mybir.AluOpType.bypass, replica_groups=GROUPS,
                    ins=[zd.opt()], outs=[zgd.opt()])
                nc.gpsimd.dma_start(
                    out=zg_sb[:].rearrange("p (k s) -> p k s", k=KC)
                        [:, 4 * hp:4 * hp + 4, qt * 512:(qt + 1) * 512],
                    in_=zgd[:, :].rearrange("(j p) s -> p j s", p=128))

            def flush_pend():
                if pend[0] is None:
                    return
                hp, qt, st, pT, kt, k0, qstart, w = pend[0]
                pend[0] = None
                q0 = qt * 512
                n_kt = 4 * qt + 4
                if kt == 0:
                    zaug_box[0] = psZ.tile([HD + 1, 512], F32, tag="z",
                                           name=f"zaug0_{hp}_{qt}")
                    zaug_box[1] = psZ.tile([HD + 1, 512], F32, tag="z",
                                           name=f"zaug1_{hp}_{qt}")
                ext = 512 + w
                nc.scalar.activation(pT[:, 0:ext], st[:, 0:ext], EXP, scale=0.125)
                if k0 >= q0:
                    for hh in range(2):
                        nc.vector.tensor_tensor(
                            pT[:, hh * 512: hh * 512 + 128],
                            pT[:, hh * 512: hh * 512 + 128],
                            tri_sb[:], MUL)
                co = qstart - q0
                vcol = kt * HPC * (HD + 1) + 2 * hp * (HD + 1)
                for hh in range(2):
                    nc.tensor.matmul(
                        zaug_box[hh][:, co:512],
                        lhsT=v_sb[:, vcol + hh * (HD + 1): vcol + (hh + 1) * (HD + 1)],
                        rhs=pT[:, hh * 512: hh * 512 + w],
                        start=(kt == 0), stop=(kt == n_kt - 1))
                if kt == n_kt - 1:
                    normalize_and_gather(hp, qt, zaug_box[0], zaug_box[1])

            def attention_qt(hp, qt, inject=None):
                q0 = qt * 512
                for kt in range(4 * qt + 4):
                    k0 = kt * 128
                    qstart = max(q0, k0)
                    w = q0 + 512 - qstart
                    st = psS.tile([128, 1024], F32, tag="s")
                    pT = ptp.tile([128, 1024], BF16, tag="pT")
                    for hh in range(2):
                        ho = hh * HD
                        nc.tensor.matmul(
                            st[:, hh * 512: hh * 512 + w],
                            lhsT=kT_sb[ho:ho + HD, hp * S + k0: hp * S + k0 + 128],
                            rhs=qT_sb[ho:ho + HD, hp * S + qstart: hp * S + qstart + w],
                            start=True, stop=True)
                    flush_pend()
                    pend[0] = (hp, qt, st, pT, kt, k0, qstart, w)
                    if inject is not None:
                        for fn in inject.get(kt, ()):
                            fn()

            ost_box = [None]

            def cproj_mt(qt, j, ks=0, ke=KC):
                if j == 0 and ks == 0:
                    ost_box[0] = obp.tile([128, 4 * ESL], F32, tag="o", name=f"ost_{qt}")
                ost = ost_box[0]
                mt = 4 * qt + j
                po = psA.tile([128, ESL], F32, tag="m")
                for k in range(ks, ke):
                    nc.tensor.matmul(
                        po[:],
                        lhsT=zg_sb[:, k * S + mt * 128: k * S + (mt + 1) * 128],
                        rhs=wp_sb[:, k * ESL:(k + 1) * ESL],
                        start=(k == ks), stop=(k == ke - 1))
                if ks == 0:
                    nc.vector.tensor_tensor(ost[:, j * ESL:(j + 1) * ESL],
                                            po[:], bp_sb[:], ADD)
                else:
                    nc.vector.tensor_tensor(ost[:, j * ESL:(j + 1) * ESL],
                                            po[:], ost[:, j * ESL:(j + 1) * ESL], ADD)
                if ke == KC:
                    nc.sync.dma_start(out=out_d[mt * 128:(mt + 1) * 128, :],
                                      in_=ost[:, j * ESL:(j + 1) * ESL])

            def cproj_qt(qt):
                for j in range(4):
                    cproj_mt(qt, j)

            for qt in range(NQT):
                if qt == 0:
                    qk_proj_nt(0, 0)
                    qk_proj_nt(2, 0)
                    for tt in range(0, 4):
                        v_proj(tt)
                n_kt = 4 * qt + 4
                inj0 = {1: [lambda q=qt: qk_proj_nt(1, q)],
                        min(3, n_kt - 1): [lambda q=qt: qk_proj_nt(3, q)]}
                if qt >= 2:
                    for j in range(4):
                        inj0.setdefault(min(4 + j, n_kt - 1), []).append(
                            lambda q=qt - 2, jj=j: cproj_mt(q, jj))
                attention_qt(0, qt, inject=inj0)
                inj1 = {}
                if qt == NQT - 1:
                    for j in range(4):
                        inj1.setdefault(min(4 + j, n_kt - 1), []).append(
                            lambda q=qt - 1, jj=j: cproj_mt(q, jj))
                    for j in range(4):
                        inj1.setdefault(min(11 + j, n_kt - 1), []).append(
                            lambda q=qt, jj=j: cproj_mt(q, jj, 0, KC // 2))
                if qt + 1 < NQT:
                    nxt = [lambda q=qt + 1: qk_proj_nt(0, q),
                           lambda q=qt + 1: qk_proj_nt(2, q)]
                    for tt in range(4 * qt + 4, 4 * qt + 8):
                        nxt.append(lambda t=tt: v_proj(t))
                    for i, fn in enumerate(nxt):
                        slot = min(n_kt - 1, max(0, n_kt - 7 + i))
                        inj1.setdefault(slot, []).append(fn)
                attention_qt(1, qt, inject=inj1)
            flush_pend()
            for j in range(4):
                cproj_mt(NQT - 1, j, KC // 2, KC)

    nc.compile()
    return nc


_NC = None


def _get_nc():
    global _NC
    if _NC is None:
        _NC = build_graph()
    return _NC


def _make_in_maps(x, w_attn, b_attn, w_proj, b_proj):
    x = np.asarray(x, dtype=np.float32)
    w_attn = np.asarray(w_attn, dtype=np.float32)
    b_attn = np.asarray(b_attn, dtype=np.float32)
    w_proj = np.asarray(w_proj, dtype=np.float32)
    b_proj = np.asarray(b_proj, dtype=np.float32)

    tri = np.triu(np.ones((128, 128), np.float32)).astype(NPBF16)
    xts = []
    for b in range(B):
        xbT = x[b].T.astype(NPBF16)
        xts.append([
            np.ascontiguousarray(
                xbT[:, nt * 512:(nt + 1) * 512]
                .reshape(KC, 128, 512).transpose(1, 0, 2).reshape(128, KC * 512))
            for nt in range(NQT)])

    in_maps = []
    for c in range(N_CORES):
        b, hg = c // TP, c % TP
        qs, ks, vs = hg * QCOLS, D + hg * QCOLS, 2 * D + hg * QCOLS
        es = (c % TP) * ESL
        wqk = np.concatenate(
            [w_attn[:, qs:qs + QCOLS], w_attn[:, ks:ks + QCOLS]], axis=1
        ).astype(NPBF16)
        wv = np.ascontiguousarray(w_attn[:, vs:vs + QCOLS]).astype(NPBF16)
        perm = np.empty(D, np.int64)
        for k in range(KC):
            hp_, j = k // TP, k % TP
            for p in range(128):
                perm[k * 128 + p] = (4 * j + 2 * hp_ + p // HD) * HD + p % HD
        wp = np.ascontiguousarray(w_proj[perm][:, es:es + ESL]).astype(NPBF16)
        bqk = np.stack([b_attn[qs:qs + 128], b_attn[qs + 128:qs + QCOLS],
                        b_attn[ks:ks + 128], b_attn[ks + 128:ks + QCOLS]],
                       axis=1).astype(np.float32)
        bv = np.ascontiguousarray(
            np.broadcast_to(b_attn[vs:vs + QCOLS], (128, QCOLS))).astype(np.float32)
        bp = np.ascontiguousarray(
            np.broadcast_to(b_proj[es:es + ESL], (128, ESL))).astype(np.float32)
        im = {
            "wqk": wqk, "wv": wv, "wp": wp,
            "bqk": bqk, "bv": bv, "bp": bp, "tri": tri,
        }
        for nt in range(NQT):
            im[f"xt{nt}"] = xts[b][nt]
        in_maps.append(im)
    return in_maps


def kernel(x, w_attn, b_attn, w_proj, b_proj):
    nc = _get_nc()
    in_maps = _make_in_maps(x, w_attn, b_attn, w_proj, b_proj)
    res = run_bass_kernel_spmd(nc, in_maps, core_ids=list(range(N_CORES)),
                               trace=bool(os.environ.get("BASS_TRACE")))
    if res.exec_time_ns is not None:
        print(f"HW exec time: {res.exec_time_ns} ns")
    out = np.empty((B, S, D), np.float32)
    for c in range(N_CORES):
        b, es = c // TP, (c % TP) * ESL
        out[b, :, es:es + ESL] = res.results[c]["out"]
    return out
